# revision 46
# baseline (speedup 1.0000x reference)
"""Trainium2 Bass kernel: 7x7 valid cross-correlation + bias on a 4096x4096 f32 image.

Formulation: banded matmul on the TensorEngine.
  out[r, c] = sum_{di,dj} w[di,dj] * x[r+di, c+dj]
For an output row-strip of M=122 rows starting at r0, using K=128 input rows:
  out[r0+m, c] = sum_k A_dj[k, m] * x[r0+k, c+dj]   summed over dj=0..6
where A_dj[k, m] = w[k-m, dj] for 0 <= k-m < 7 (a banded [128, 122] matrix,
precomputed on host from the 49 kernel weights). The 7 dj-terms accumulate
into one PSUM bank via shifted column slices of the same SBUF rhs tile.

All matmul operands are fp16 (1 cycle/row on the PE vs fp32's 4), PSUM
accumulates fp32, output staged to SBUF as fp16 and upcast on the host.

DMA strategy: each dma_start costs ~1.2us of sequencer time regardless of
size, and PE stalls reset the p-state ramp (2.4GHz only after 3us of
continuous execution). So the host pre-arranges the input strip-major as
xst[p, s, c] = x[122*s + p, c0 + c] and the kernel loads it in a few large
chunk DMAs on the SP queue; outputs are written strip-major to
outt[m, s, c] = out[122*s + m, c0 + c] in one batched DMA per weight-group
on the Activation queue. The PE then streams matmuls back-to-back.

Sharding: output columns are split across the 8 cores (512 cols/core);
each core processes all 4090 output rows. Kernel + bias replicated.
"""

import numpy as np

H, W = 4096, 4096
KH, KW = 7, 7
OH, OW = H - KH + 1, W - KW + 1  # 4090, 4090
N_CORES = 8
CW = 512               # output columns per core
IW = CW + KW - 1       # input columns per core (518)
STRIP = 122            # output rows per strip (K = STRIP + KH - 1 = 128)
MB = 128               # stationary block columns (M padded 122 -> 128)
N_STRIPS = (OH + STRIP - 1) // STRIP  # 34 (last strip M=64, K=70)
CHUNK_SIZES = (2, 4, 6, 8, 8, 6)  # input DMA chunks; small first = early start
N_WARM = 12            # junk matmuls at t=0: ramp the PE p-state clock and
                       # bridge until the first input chunk lands (~12us)


def _regions():
    """Output flush units: (start_strip, n_strips, queue, flat_offset).

    One unit per strip: the PSUM accumulator is cast-DMA'd (f32 -> fp16)
    straight to a contiguous DRAM region by the SWDGE (gpsimd) queue.
    Reading PSUM avoids SBUF port contention with the PE's rhs streaming,
    and contiguous destinations let the DMA engines spread descriptors.
    """
    return [
        (s, 1, "g" if s % 2 == 0 else "s", s * STRIP * CW)
        for s in range(N_STRIPS)
    ]

_cache = {}


def _chunks():
    s0 = 0
    out = []
    for ns in CHUNK_SIZES:
        ns = min(ns, N_STRIPS - s0)
        if ns <= 0:
            break
        out.append((s0, ns))
        s0 += ns
    assert s0 == N_STRIPS, (s0, N_STRIPS)
    return out


def _build_nc():
    import concourse.bacc as bacc
    import concourse.mybir as mybir
    from concourse.tile import TileContext

    f16 = mybir.dt.float16
    f32 = mybir.dt.float32

    n_chunks = len(_chunks())

    nc = bacc.Bacc("TRN2", target_bir_lowering=False, debug=False)
    xst = nc.dram_tensor("xst", [128, N_STRIPS, IW], f16, kind="ExternalInput")
    bands = nc.dram_tensor("bands", [128, KW * MB], f16, kind="ExternalInput")
    outt = nc.dram_tensor("outt", [STRIP * N_STRIPS * CW], f16, kind="ExternalOutput")

    with TileContext(nc) as tc:
        with (
            tc.tile_pool(name="const", bufs=1) as cpool,
            tc.tile_pool(name="rhs", bufs=1) as rpool,
            tc.tile_pool(name="obuf", bufs=6) as opool,
            tc.tile_pool(name="psum", bufs=8, space="PSUM") as ppool,
        ):
            # PE clock warmup: junk matmuls keep the PE continuously busy from
            # t=0 so the p-state ramp (0.65 -> 1.2 -> 2.4 GHz after 3us of
            # continuous execution) completes while the first input chunk DMAs.
            warm_t = cpool.tile([128, MB + CW], f16)
            nc.vector.memset(warm_t[:, :], 0.0)
            wps = ppool.tile([128, CW], f32, name="wps", tag="ps")
            for _ in range(N_WARM):
                nc.tensor.matmul(
                    wps[:, :],
                    warm_t[:, :MB],
                    warm_t[:, MB : MB + CW],
                    start=True,
                    stop=True,
                )

            band_t = cpool.tile([128, KW * MB], f16)
            nc.scalar.dma_start(out=band_t[:, :], in_=bands[:, :])

            chunk_map = {}
            for ci, (s0, ns) in enumerate(_chunks()):
                ct = rpool.tile([128, ns * IW], f16, tag=f"rhs{ci}")
                nc.sync.dma_start(
                    out=ct[:, : ns * IW], in_=xst[:, s0 : s0 + ns, :]
                )
                for i in range(ns):
                    chunk_map[s0 + i] = (ct, i * IW)

            for s in range(N_STRIPS):
                r0 = s * STRIP
                K = min(128, H - r0)
                ct, off = chunk_map[s]
                ps = ppool.tile([128, CW], f32, name="ps", tag="ps")
                for dj in range(KW):
                    nc.tensor.matmul(
                        ps[:, :],
                        band_t[:K, dj * MB : dj * MB + MB],
                        ct[:K, off + dj : off + dj + CW],
                        start=(dj == 0),
                        stop=(dj == KW - 1),
                    )
                ot = opool.tile([128, CW], f16, name="ot", tag="ot")
                nc.vector.tensor_copy(ot[:STRIP, :], ps[:STRIP, :])
                dst = outt[
                    s * STRIP * CW : (s + 1) * STRIP * CW
                ].rearrange("(m c) -> m c", m=STRIP)
                eng = nc.gpsimd if s % 2 == 0 else nc.scalar
                eng.dma_start(out=dst, in_=ot[:STRIP, :])

    nc.finalize()
    return nc


def _get_nc():
    if "nc" not in _cache:
        _cache["nc"] = _build_nc()
    return _cache["nc"]


def _build_bands(weight: np.ndarray) -> np.ndarray:
    """bands[k, dj*MB + m] = weight[k - m, dj] for 0 <= k-m < KH, m < STRIP."""
    w = np.asarray(weight, np.float32)
    bands = np.zeros((128, KW * MB), np.float32)
    m = np.arange(STRIP)
    for dj in range(KW):
        for di in range(KH):
            bands[m + di, dj * MB + m] = w[di, dj]
    return bands.astype(np.float16)


def _prepare_in_maps(x, weight, bias):
    x16 = np.asarray(x, np.float32).astype(np.float16)
    bands = _build_bands(weight)

    # padded copy: rows up to 122*33+127, cols up to 7*512+517
    rmax = STRIP * (N_STRIPS - 1) + 128
    cmax = CW * (N_CORES - 1) + IW
    xp = np.zeros((rmax, cmax), np.float16)
    xp[:H, :W] = x16
    rows = STRIP * np.arange(N_STRIPS)[None, :] + np.arange(128)[:, None]  # [128, S]

    in_maps = []
    for c in range(N_CORES):
        c0 = c * CW
        blk = xp[:, c0 : c0 + IW]          # [rmax, IW]
        xst = np.ascontiguousarray(blk[rows])  # [128, S, IW]
        in_maps.append({"xst": xst, "bands": bands})
    return in_maps


def _gather_out(per_core_outs) -> np.ndarray:
    regions = _regions()
    out = np.empty((OH, OW), np.float32)
    rows = np.empty((N_STRIPS * STRIP, CW), np.float16)
    for c in range(N_CORES):
        c0 = c * CW
        take = min(CW, OW - c0)
        flat = per_core_outs[c]["outt"]  # [STRIP*S*CW] fp16, region-major
        for rs0, rns, _q, roff in regions:
            blk = flat[roff : roff + STRIP * rns * CW].reshape(STRIP, rns, CW)
            rows[rs0 * STRIP : (rs0 + rns) * STRIP] = (
                blk.transpose(1, 0, 2).reshape(rns * STRIP, CW)
            )
        out[:, c0 : c0 + take] = rows[:OH, :take].astype(np.float32)
    return out


def kernel(x: np.ndarray, weight: np.ndarray, bias: np.ndarray) -> np.ndarray:
    from concourse import bass_utils

    nc = _get_nc()
    in_maps = _prepare_in_maps(x, weight, bias)
    res = bass_utils.run_bass_kernel_spmd(nc, in_maps, list(range(N_CORES)))
    _cache["last_results"] = res
    out = _gather_out(res.results)
    b = np.float32(np.asarray(bias, np.float32).reshape(-1)[0])
    if b != 0.0:
        out += b
    return out


# revision 50
# speedup vs baseline: 1.0787x; 1.0787x over previous
"""Trainium2 Bass kernel: 7x7 valid cross-correlation + bias on a 4096x4096 f32 image.

Formulation: banded matmul on the TensorEngine.
  out[r, c] = sum_{di,dj} w[di,dj] * x[r+di, c+dj]
For an output row-strip of M=122 rows starting at r0, using K=128 input rows:
  out[r0+m, c] = sum_k A_dj[k, m] * x[r0+k, c+dj]   summed over dj=0..6
where A_dj[k, m] = w[k-m, dj] for 0 <= k-m < 7 (a banded [128, 122] matrix,
precomputed on host from the 49 kernel weights). The 7 dj-terms accumulate
into one PSUM bank via shifted column slices of the same SBUF rhs tile.

All matmul operands are fp16 (1 cycle/row on the PE vs fp32's 4), PSUM
accumulates fp32, output staged to SBUF as fp16 and upcast on the host.

DMA strategy: each dma_start costs ~1.2us of sequencer time regardless of
size, and PE stalls reset the p-state ramp (2.4GHz only after 3us of
continuous execution). So the host pre-arranges the input strip-major as
xst[p, s, c] = x[122*s + p, c0 + c] and the kernel loads it in a few large
chunk DMAs on the SP queue; outputs are written strip-major to
outt[m, s, c] = out[122*s + m, c0 + c] in one batched DMA per weight-group
on the Activation queue. The PE then streams matmuls back-to-back.

Sharding: output columns are split across the 8 cores (512 cols/core);
each core processes all 4090 output rows. Kernel + bias replicated.
"""

import numpy as np

H, W = 4096, 4096
KH, KW = 7, 7
OH, OW = H - KH + 1, W - KW + 1  # 4090, 4090
N_CORES = 8
CW = 512               # output columns per core
IW = CW + KW - 1       # input columns per core (518)
STRIP = 122            # output rows per strip (K = STRIP + KH - 1 = 128)
MB = 128               # stationary block columns (M padded 122 -> 128)
N_STRIPS = (OH + STRIP - 1) // STRIP  # 34 (last strip M=64, K=70)
CHUNK_SIZES = (2, 4, 6, 8, 8, 6)  # input DMA chunks; small first = early start
OUT_BATCHES = (8, 8, 8, 6, 2, 1, 1)  # strips per obuf staging tile
N_WARM = 12            # junk matmuls at t=0: ramp the PE p-state clock and
                       # bridge until the first input chunk lands (~12us)


def _regions():
    """Output flush units: (start_strip, n_strips, queue, flat_offset).

    Each unit writes one fully CONTIGUOUS DRAM region laid out [m, s, c]
    (m = row within strip). Contiguous destinations let the DMA engines
    spread/merge descriptors; big batches split 2:1 across the SWDGE
    (gpsimd) and HWDGE (scalar) queues so both paths drain in parallel.
    """
    regs = []
    off = 0
    b0 = 0
    for nb in OUT_BATCHES:
        subs = [(0, (2 * nb + 2) // 3, "g")] if nb >= 4 else [(0, nb, "g")]
        if subs[0][1] < nb:
            subs.append((subs[0][1], nb, "s"))
        for a, b, q in subs:
            regs.append((b0 + a, b - a, q, off))
            off += STRIP * (b - a) * CW
        b0 += nb
    return regs

_cache = {}


def _chunks():
    s0 = 0
    out = []
    for ns in CHUNK_SIZES:
        ns = min(ns, N_STRIPS - s0)
        if ns <= 0:
            break
        out.append((s0, ns))
        s0 += ns
    assert s0 == N_STRIPS, (s0, N_STRIPS)
    return out


def _build_nc():
    import concourse.bacc as bacc
    import concourse.mybir as mybir
    from concourse.tile import TileContext

    f16 = mybir.dt.float16
    f32 = mybir.dt.float32

    n_chunks = len(_chunks())

    nc = bacc.Bacc("TRN2", target_bir_lowering=False, debug=False)
    xst = nc.dram_tensor("xst", [128, N_STRIPS, IW], f16, kind="ExternalInput")
    bands = nc.dram_tensor("bands", [128, KW * MB], f16, kind="ExternalInput")
    outt = nc.dram_tensor("outt", [STRIP * N_STRIPS * CW], f16, kind="ExternalOutput")

    with TileContext(nc) as tc:
        with (
            tc.tile_pool(name="const", bufs=1) as cpool,
            tc.tile_pool(name="rhs", bufs=1) as rpool,
            tc.tile_pool(name="obuf", bufs=4) as opool,
            tc.tile_pool(name="psum", bufs=8, space="PSUM") as ppool,
        ):
            # PE clock warmup: junk matmuls keep the PE continuously busy from
            # t=0 so the p-state ramp (0.65 -> 1.2 -> 2.4 GHz after 3us of
            # continuous execution) completes while the first input chunk DMAs.
            warm_t = cpool.tile([128, MB + CW], f16)
            nc.vector.memset(warm_t[:, :], 0.0)
            wps = ppool.tile([128, CW], f32, name="wps", tag="ps")
            for _ in range(N_WARM):
                nc.tensor.matmul(
                    wps[:, :],
                    warm_t[:, :MB],
                    warm_t[:, MB : MB + CW],
                    start=True,
                    stop=True,
                )

            band_t = cpool.tile([128, KW * MB], f16)
            nc.scalar.dma_start(out=band_t[:, :], in_=bands[:, :])

            chunk_map = {}
            for ci, (s0, ns) in enumerate(_chunks()):
                ct = rpool.tile([128, ns * IW], f16, tag=f"rhs{ci}")
                nc.sync.dma_start(
                    out=ct[:, : ns * IW], in_=xst[:, s0 : s0 + ns, :]
                )
                for i in range(ns):
                    chunk_map[s0 + i] = (ct, i * IW)

            # strip -> (staging tile, column offset, batch start, size)
            obuf_map = {}
            b0 = 0
            for bi, nb in enumerate(OUT_BATCHES):
                if nb >= 4:
                    ot = opool.tile(
                        [128, max(OUT_BATCHES) * CW], f16, name="ot", tag="ot"
                    )
                else:
                    # tail batches get dedicated tiles: no WAR wait on a slow
                    # prior write draining from the rotation slots
                    ot = cpool.tile([128, nb * CW], f16, name=f"ot_tail{bi}")
                for i in range(nb):
                    obuf_map[b0 + i] = (ot, i * CW, b0, nb)
                b0 += nb

            regions = _regions()
            for s in range(N_STRIPS):
                r0 = s * STRIP
                K = min(128, H - r0)
                ct, off = chunk_map[s]
                ps = ppool.tile([128, CW], f32, name="ps", tag="ps")
                for dj in range(KW):
                    nc.tensor.matmul(
                        ps[:, :],
                        band_t[:K, dj * MB : dj * MB + MB],
                        ct[:K, off + dj : off + dj + CW],
                        start=(dj == 0),
                        stop=(dj == KW - 1),
                    )
                ot, coff, ob0, nb = obuf_map[s]
                nc.vector.tensor_copy(
                    ot[:STRIP, coff : coff + CW], ps[:STRIP, :]
                )
                if s == ob0 + nb - 1:  # batch complete -> flush its regions
                    for rs0, rns, q, roff in regions:
                        if not (ob0 <= rs0 < ob0 + nb):
                            continue
                        dst = outt[
                            roff : roff + STRIP * rns * CW
                        ].rearrange("(m s c) -> m s c", m=STRIP, s=rns)
                        a = rs0 - ob0
                        eng = nc.gpsimd if q == "g" else nc.scalar
                        eng.dma_start(
                            out=dst, in_=ot[:STRIP, a * CW : (a + rns) * CW]
                        )

    nc.finalize()
    return nc


def _get_nc():
    if "nc" not in _cache:
        _cache["nc"] = _build_nc()
    return _cache["nc"]


def _build_bands(weight: np.ndarray) -> np.ndarray:
    """bands[k, dj*MB + m] = weight[k - m, dj] for 0 <= k-m < KH, m < STRIP."""
    w = np.asarray(weight, np.float32)
    bands = np.zeros((128, KW * MB), np.float32)
    m = np.arange(STRIP)
    for dj in range(KW):
        for di in range(KH):
            bands[m + di, dj * MB + m] = w[di, dj]
    return bands.astype(np.float16)


def _prepare_in_maps(x, weight, bias):
    x16 = np.asarray(x, np.float32).astype(np.float16)
    bands = _build_bands(weight)

    # padded copy: rows up to 122*33+127, cols up to 7*512+517
    rmax = STRIP * (N_STRIPS - 1) + 128
    cmax = CW * (N_CORES - 1) + IW
    xp = np.zeros((rmax, cmax), np.float16)
    xp[:H, :W] = x16
    rows = STRIP * np.arange(N_STRIPS)[None, :] + np.arange(128)[:, None]  # [128, S]

    in_maps = []
    for c in range(N_CORES):
        c0 = c * CW
        blk = xp[:, c0 : c0 + IW]          # [rmax, IW]
        xst = np.ascontiguousarray(blk[rows])  # [128, S, IW]
        in_maps.append({"xst": xst, "bands": bands})
    return in_maps


def _gather_out(per_core_outs) -> np.ndarray:
    regions = _regions()
    out = np.empty((OH, OW), np.float32)
    rows = np.empty((N_STRIPS * STRIP, CW), np.float16)
    for c in range(N_CORES):
        c0 = c * CW
        take = min(CW, OW - c0)
        flat = per_core_outs[c]["outt"]  # [STRIP*S*CW] fp16, region-major
        for rs0, rns, _q, roff in regions:
            blk = flat[roff : roff + STRIP * rns * CW].reshape(STRIP, rns, CW)
            rows[rs0 * STRIP : (rs0 + rns) * STRIP] = (
                blk.transpose(1, 0, 2).reshape(rns * STRIP, CW)
            )
        out[:, c0 : c0 + take] = rows[:OH, :take].astype(np.float32)
    return out


def kernel(x: np.ndarray, weight: np.ndarray, bias: np.ndarray) -> np.ndarray:
    from concourse import bass_utils

    nc = _get_nc()
    in_maps = _prepare_in_maps(x, weight, bias)
    res = bass_utils.run_bass_kernel_spmd(nc, in_maps, list(range(N_CORES)))
    _cache["last_results"] = res
    out = _gather_out(res.results)
    b = np.float32(np.asarray(bias, np.float32).reshape(-1)[0])
    if b != 0.0:
        out += b
    return out
